# revision 16
# baseline (speedup 1.0000x reference)
"""Memristor linear layer kernel for 8 TRN2 NeuronCores.

The reference memristor crossbar computation collapses algebraically to
    out = x @ weights.T + bias
(the G_OFF offsets cancel in the pos/neg column subtraction and the k_G /
k_I scale factors cancel exactly), so the kernel computes the plain linear
layer.

Precision: fp32 operands are split on host into bf16 hi + bf16 lo halves;
the device computes hi*hi + hi*lo + lo*hi with fp32 PSUM accumulation
(~4e-6 relative error vs 3e-7 for native fp32) at full bf16 PE rate.

Sharding: tensor-parallel over the 1024 output features -> 128 per core.
Each core receives x.T (replicated) and its W.T column shard, pre-packed
on host into the exact SBUF layout [128 partitions, k_tile, free] so
every DMA moves per-partition-contiguous rows at line rate. Each core
computes its out.T shard [128, 256] = W_shard @ x.T + bias accumulated
over 8 K-chunks of 128 in PSUM. Host concatenates and transposes back.
"""

import os

import numpy as np

BATCH = 256
SIZE_IN = 1024
SIZE_OUT = 1024
N_CORES = 8
O_SHARD = SIZE_OUT // N_CORES  # 128
K_TILES = SIZE_IN // 128  # 8

_STATE = {}


def _build():
    import concourse.bass as bass
    import concourse.tile as tile
    from concourse import bacc, mybir

    f32 = mybir.dt.float32
    bf16 = mybir.dt.bfloat16
    n_warm = int(os.environ.get("WARMUP_MM", "6"))

    nc = bacc.Bacc(None, target_bir_lowering=False)

    # All tensors pre-packed on host to [128, ..., free] (partition major)
    # so every DMA descriptor is a large per-partition-contiguous run.
    xh_d = nc.declare_dram_parameter("xh", [128, K_TILES, BATCH], bf16, isOutput=False)
    xl_d = nc.declare_dram_parameter("xl", [128, K_TILES, BATCH], bf16, isOutput=False)
    whl_d = nc.declare_dram_parameter(
        "whl", [128, 2, K_TILES, O_SHARD], bf16, isOutput=False
    )
    b_d = nc.declare_dram_parameter("bias", [O_SHARD, 1], f32, isOutput=False)
    out_d = nc.declare_dram_parameter("out", [O_SHARD, BATCH], f32, isOutput=True)

    with tile.TileContext(nc) as tc:
        with (
            tc.tile_pool(name="sbuf", bufs=1) as pool,
            tc.tile_pool(name="psum", bufs=1, space="PSUM") as psum_pool,
        ):
            xh_s = pool.tile([128, K_TILES, BATCH], bf16)
            xl_s = pool.tile([128, K_TILES, BATCH], bf16)
            whl_s = pool.tile([128, 2, K_TILES, O_SHARD], bf16)
            b_s = pool.tile([O_SHARD, 1], f32)
            o_s = pool.tile([O_SHARD, BATCH], f32)
            pt = psum_pool.tile([O_SHARD, BATCH], f32)

            # PE warm-up: garbage matmuls into a scratch PSUM bank so the
            # HAM clock-gate releases (1.2 -> 2.4 GHz) while DMAs stream.
            # A few big ones build the busy window, then small ones keep PE
            # occupied at fine granularity until real data lands.
            n_warm_small = int(os.environ.get("WARMUP_MM_SMALL", "10"))
            if n_warm or n_warm_small:
                warm_in = pool.tile([128, 512], bf16)
                warm_ps = psum_pool.tile([128, 512], f32)
                nc.vector.memset(warm_in[:], 0.0)
                for _ in range(n_warm):
                    nc.tensor.matmul(
                        warm_ps[:], warm_in[:, 0:128], warm_in[:], start=True,
                        stop=True,
                    )
                for _ in range(n_warm_small):
                    nc.tensor.matmul(
                        warm_ps[:, 0:64], warm_in[:, 0:128], warm_in[:, 0:64],
                        start=True, stop=True,
                    )

            # Fine-grained transfers. Each engine issues its own queue in
            # program order and the HWDGE drains in global issue-time
            # order, so keep everything whose order matters on the scalar
            # ring; sync carries only the two wh halves issued up front.
            h = K_TILES // 2
            q = K_TILES // 4
            nc.sync.dma_start(out=whl_s[:, 0, 0:h, :], in_=whl_d[:, 0, 0:h, :])
            nc.sync.dma_start(out=whl_s[:, 0, h:, :], in_=whl_d[:, 0, h:, :])
            nc.scalar.dma_start(out=xh_s[:, 0:q, :], in_=xh_d[:, 0:q, :])
            nc.scalar.dma_start(out=xh_s[:, q : 2 * q, :], in_=xh_d[:, q : 2 * q, :])
            nc.scalar.dma_start(out=xh_s[:, 2 * q : 3 * q, :], in_=xh_d[:, 2 * q : 3 * q, :])
            nc.scalar.dma_start(out=xh_s[:, 3 * q :, :], in_=xh_d[:, 3 * q :, :])
            nc.scalar.dma_start(out=whl_s[:, 1, :, :], in_=whl_d[:, 1, :, :])
            nc.scalar.dma_start(out=xl_s[:, 0:h, :], in_=xl_d[:, 0:h, :])
            nc.scalar.dma_start(out=xl_s[:, h : h + q, :], in_=xl_d[:, h : h + q, :])
            nc.scalar.dma_start(out=xl_s[:, h + q :, :], in_=xl_d[:, h + q :, :])
            # bias via the gpsimd SWDGE path (off both HWDGE rings)
            nc.gpsimd.dma_start(out=b_s[:], in_=b_d[:])

            # 24 accumulating matmuls ordered to chase the DMA stream:
            # hi*hi k0..7 (chasing xh quarters), lo*hi k0..7, hi*lo k0..7.
            plan = (
                [(0, xh_s, k) for k in range(K_TILES)]
                + [(1, xh_s, k) for k in range(K_TILES)]
                + [(0, xl_s, k) for k in range(K_TILES)]
            )
            for i, (hl, xs, k) in enumerate(plan):
                nc.tensor.matmul(
                    pt[:],
                    whl_s[:, hl, k, :],
                    xs[:, k, :],
                    start=(i == 0),
                    stop=(i == len(plan) - 1),
                )

            nc.vector.tensor_scalar_add(out=o_s[:], in0=pt[:], scalar1=b_s[:])
            # out halves on both HWDGE rings so the completion receipts
            # (~1 us each to HBM) overlap
            nc.sync.dma_start(out=out_d[:, 0:BATCH // 2], in_=o_s[:, 0:BATCH // 2])
            nc.scalar.dma_start(out=out_d[:, BATCH // 2 :], in_=o_s[:, BATCH // 2 :])

    nc.compile()
    return nc


def _install_ntff_hook_shim():
    """The agent image's antenv lacks axon_hooks; recreate it so
    run_bass_kernel_spmd(trace=True) can capture NTFF profiles."""
    import sys
    import types

    if "antenv.axon_hooks" in sys.modules:
        return
    mod = types.ModuleType("antenv.axon_hooks")
    mod._HOOK = None

    def set_axon_ntff_profile_hook(hook):
        mod._HOOK = hook

    def get_axon_ntff_profile_hook():
        return mod._HOOK

    mod.set_axon_ntff_profile_hook = set_axon_ntff_profile_hook
    mod.get_axon_ntff_profile_hook = get_axon_ntff_profile_hook
    sys.modules["antenv.axon_hooks"] = mod
    try:
        from trn_agent_boot.trn_boot import _ntff_profile_via_ctypes

        mod._HOOK = _ntff_profile_via_ctypes("/opt/axon/libaxon_pjrt.so")
    except Exception:
        pass


def _split_pack(a_t: np.ndarray, ncols: int):
    """[SIZE_IN, ncols] f32 -> two bf16 arrays packed as [128, K_TILES, ncols]."""
    import ml_dtypes

    hi = a_t.astype(ml_dtypes.bfloat16)
    lo = (a_t - hi.astype(np.float32)).astype(ml_dtypes.bfloat16)

    def pack(v):
        return np.ascontiguousarray(
            v.reshape(K_TILES, 128, ncols).transpose(1, 0, 2)
        )

    return pack(hi), pack(lo)


def _split_pack_w(w_t: np.ndarray):
    """[SIZE_IN, O_SHARD] f32 -> one bf16 array [128, 2, K_TILES, O_SHARD]
    holding the hi and lo halves contiguously per partition."""
    hi, lo = _split_pack(w_t, O_SHARD)
    return np.ascontiguousarray(np.stack([hi, lo], axis=1))


def kernel(x: np.ndarray, weights: np.ndarray, bias: np.ndarray) -> np.ndarray:
    from concourse.bass_utils import run_bass_kernel_spmd

    if "nc" not in _STATE:
        _STATE["nc"] = _build()
    nc = _STATE["nc"]

    x = np.asarray(x, dtype=np.float32)
    weights = np.asarray(weights, dtype=np.float32)
    bias = np.asarray(bias, dtype=np.float32)

    xt = np.ascontiguousarray(x.T)  # [SIZE_IN, BATCH] f32
    xh, xl = _split_pack(xt, BATCH)
    wt = np.ascontiguousarray(weights.T)  # [SIZE_IN, SIZE_OUT] f32

    in_maps = []
    for c in range(N_CORES):
        sl = slice(c * O_SHARD, (c + 1) * O_SHARD)
        in_maps.append(
            {
                "xh": xh,
                "xl": xl,
                "whl": _split_pack_w(np.ascontiguousarray(wt[:, sl])),
                "bias": np.ascontiguousarray(bias[sl]).reshape(O_SHARD, 1),
            }
        )

    trace = os.environ.get("BASS_PROBLEM_TRACE", "0") == "1"
    if trace:
        _install_ntff_hook_shim()
    res = run_bass_kernel_spmd(
        nc, in_maps, core_ids=list(range(N_CORES)), trace=trace
    )
    _STATE["last_results"] = res

    out_t = np.concatenate(
        [np.asarray(res.results[c]["out"]) for c in range(N_CORES)], axis=0
    )  # [SIZE_OUT, BATCH]
    return np.ascontiguousarray(out_t.T).astype(np.float32, copy=False)


# revision 26
# speedup vs baseline: 1.0595x; 1.0595x over previous
"""Memristor linear layer kernel for 8 TRN2 NeuronCores.

The reference memristor crossbar computation collapses algebraically to
    out = x @ weights.T + bias
(the G_OFF offsets cancel in the pos/neg column subtraction and the k_G /
k_I scale factors cancel exactly), so the kernel computes the plain linear
layer.

Precision: fp32 operands are split on host into bf16 hi + bf16 lo halves;
the device computes hi*hi + hi*lo + lo*hi with fp32 PSUM accumulation
(~4e-6 relative error vs 3e-7 for native fp32) at full bf16 PE rate.

Sharding: tensor-parallel over the 1024 output features -> 128 per core.
Each core receives x.T (replicated) and its W.T column shard, pre-packed
on host into the exact SBUF layout [128 partitions, k_tile, free] so
every DMA moves per-partition-contiguous rows at line rate. Each core
computes its out.T shard [128, 256] = W_shard @ x.T + bias accumulated
over 8 K-chunks of 128 in PSUM. Host concatenates and transposes back.

Schedule notes (from NTFF profiling on TRN2 under axon):
- The HWDGE rings drain in global issue order at ~280 GB/s, with ~1 us
  per-transfer completion latency, so transfers are staged in the exact
  order the matmul passes need them (wh | xh halves, wl, xl halves).
- The PE HAM clock gate needs ~3.4 us of sustained busy-ness to release
  (1.2 -> 2.4 GHz) and re-throttles after ~2 us of idle, so garbage
  warm-up matmuls run while DMAs stream and tiny filler matmuls are
  interleaved between compute passes to bridge DMA chase-stalls.
"""

import os

import numpy as np

BATCH = 256
SIZE_IN = 1024
SIZE_OUT = 1024
N_CORES = 8
O_SHARD = SIZE_OUT // N_CORES  # 128
K_TILES = SIZE_IN // 128  # 8

_STATE = {}


def _build():
    import concourse.bass as bass
    import concourse.tile as tile
    from concourse import bacc, mybir

    f32 = mybir.dt.float32
    bf16 = mybir.dt.bfloat16
    n_warm = int(os.environ.get("WARMUP_MM", "5"))

    nc = bacc.Bacc(None, target_bir_lowering=False)

    # All tensors pre-packed on host to [128, ..., free] (partition major)
    # so every DMA descriptor is a large per-partition-contiguous run.
    xh_d = nc.declare_dram_parameter("xh", [128, K_TILES, BATCH], bf16, isOutput=False)
    xl_d = nc.declare_dram_parameter("xl", [128, K_TILES, BATCH], bf16, isOutput=False)
    whl_d = nc.declare_dram_parameter(
        "whl", [128, 2, K_TILES, O_SHARD], bf16, isOutput=False
    )
    b_d = nc.declare_dram_parameter("bias", [O_SHARD, 1], f32, isOutput=False)
    out_d = nc.declare_dram_parameter("out", [O_SHARD, BATCH], f32, isOutput=True)

    with tile.TileContext(nc) as tc:
        with (
            tc.tile_pool(name="sbuf", bufs=1) as pool,
            tc.tile_pool(name="psum", bufs=1, space="PSUM") as psum_pool,
        ):
            xh_s = pool.tile([128, K_TILES, BATCH], bf16)
            xl_s = pool.tile([128, K_TILES, BATCH], bf16)
            whl_s = pool.tile([128, 2, K_TILES, O_SHARD], bf16)
            b_s = pool.tile([O_SHARD, 1], f32)
            o_s = pool.tile([O_SHARD, BATCH], f32)
            pt = psum_pool.tile([O_SHARD, BATCH], f32)

            # PE warm-up: garbage matmuls into a scratch PSUM bank so the
            # HAM clock-gate releases (1.2 -> 2.4 GHz) while DMAs stream.
            # A few big ones build the busy window, then small (~54 ns)
            # ones keep PE occupied at fine granularity until real data
            # lands; more small ones are interleaved between the compute
            # passes below so DMA chase-stalls can't re-throttle the PE.
            n_warm_small = int(os.environ.get("WARMUP_MM_SMALL", "10"))
            warm_in = pool.tile([128, 512], bf16)
            warm_ps = psum_pool.tile([128, 512], f32)
            nc.vector.memset(warm_in[:], 0.0)

            def warm_big(n):
                for _ in range(n):
                    nc.tensor.matmul(
                        warm_ps[:], warm_in[:, 0:128], warm_in[:], start=True,
                        stop=True,
                    )

            def warm_small(n):
                for _ in range(n):
                    nc.tensor.matmul(
                        warm_ps[:, 0:64], warm_in[:, 0:128], warm_in[:, 0:64],
                        start=True, stop=True,
                    )

            warm_big(n_warm)
            warm_small(n_warm_small)

            # Fine-grained transfers. Each engine issues its own queue in
            # program order and the HWDGE drains in global issue-time
            # order, so keep everything whose order matters on the scalar
            # ring; sync carries only the two wh halves issued up front.
            h = K_TILES // 2
            q = K_TILES // 4
            granularity = os.environ.get("DMA_GRAN", "half")
            nc.sync.dma_start(out=whl_s[:, 0, :, :], in_=whl_d[:, 0, :, :])
            if granularity == "quarter":
                for j in range(4):
                    nc.scalar.dma_start(
                        out=xh_s[:, j * q : (j + 1) * q, :],
                        in_=xh_d[:, j * q : (j + 1) * q, :],
                    )
            elif granularity == "full":
                nc.scalar.dma_start(out=xh_s[:], in_=xh_d[:])
            else:
                nc.scalar.dma_start(out=xh_s[:, 0:h, :], in_=xh_d[:, 0:h, :])
                nc.scalar.dma_start(out=xh_s[:, h:, :], in_=xh_d[:, h:, :])
            nc.scalar.dma_start(out=whl_s[:, 1, :, :], in_=whl_d[:, 1, :, :])
            if granularity == "full":
                nc.scalar.dma_start(out=xl_s[:], in_=xl_d[:])
            else:
                nc.scalar.dma_start(out=xl_s[:, 0:h, :], in_=xl_d[:, 0:h, :])
                nc.scalar.dma_start(out=xl_s[:, h:, :], in_=xl_d[:, h:, :])
            # bias via the gpsimd SWDGE path (off both HWDGE rings)
            nc.gpsimd.dma_start(out=b_s[:], in_=b_d[:])

            # 24 accumulating matmuls ordered to chase the DMA stream:
            # hi*hi k0..7 (chasing xh halves), lo*hi k0..7, hi*lo k0..7.
            # None marks a gap where the DMA stream may not have caught up
            # yet; small warm matmuls are inserted to keep the PE busy.
            plan = (
                [(0, xh_s, k) for k in range(h)]
                + [None]
                + [(0, xh_s, k) for k in range(h, K_TILES)]
                + [None]
                + [(1, xh_s, k) for k in range(K_TILES)]
                + [None]
                + [(0, xl_s, k) for k in range(K_TILES)]
            )
            mm_plan = [p for p in plan if p is not None]
            i = 0
            for p in plan:
                if p is None:
                    warm_small(int(os.environ.get("WARMUP_MM_GAP", "8")))
                    continue
                hl, xs, k = p
                nc.tensor.matmul(
                    pt[:],
                    whl_s[:, hl, k, :],
                    xs[:, k, :],
                    start=(i == 0),
                    stop=(i == len(mm_plan) - 1),
                )
                i += 1

            nc.vector.tensor_scalar_add(out=o_s[:], in0=pt[:], scalar1=b_s[:])
            # out halves on both HWDGE rings so the completion receipts
            # (~1 us each to HBM) overlap
            nc.sync.dma_start(out=out_d[:, 0:BATCH // 2], in_=o_s[:, 0:BATCH // 2])
            nc.scalar.dma_start(out=out_d[:, BATCH // 2 :], in_=o_s[:, BATCH // 2 :])

    nc.compile()
    return nc


def _install_ntff_hook_shim():
    """The agent image's antenv lacks axon_hooks; recreate it so
    run_bass_kernel_spmd(trace=True) can capture NTFF profiles."""
    import sys
    import types

    if "antenv.axon_hooks" in sys.modules:
        return
    try:
        import antenv.axon_hooks  # noqa: F401  (real module exists)

        return
    except ImportError:
        pass
    mod = types.ModuleType("antenv.axon_hooks")
    mod._HOOK = None

    def set_axon_ntff_profile_hook(hook):
        mod._HOOK = hook

    def get_axon_ntff_profile_hook():
        return mod._HOOK

    mod.set_axon_ntff_profile_hook = set_axon_ntff_profile_hook
    mod.get_axon_ntff_profile_hook = get_axon_ntff_profile_hook
    sys.modules["antenv.axon_hooks"] = mod
    try:
        from trn_agent_boot.trn_boot import _ntff_profile_via_ctypes

        mod._HOOK = _ntff_profile_via_ctypes("/opt/axon/libaxon_pjrt.so")
    except Exception:
        pass


def _split_pack(a_t: np.ndarray, ncols: int):
    """[SIZE_IN, ncols] f32 -> two bf16 arrays packed as [128, K_TILES, ncols]."""
    import ml_dtypes

    hi = a_t.astype(ml_dtypes.bfloat16)
    lo = (a_t - hi.astype(np.float32)).astype(ml_dtypes.bfloat16)

    def pack(v):
        return np.ascontiguousarray(
            v.reshape(K_TILES, 128, ncols).transpose(1, 0, 2)
        )

    return pack(hi), pack(lo)


def _split_pack_w(w_t: np.ndarray):
    """[SIZE_IN, O_SHARD] f32 -> one bf16 array [128, 2, K_TILES, O_SHARD]
    holding the hi and lo halves contiguously per partition."""
    hi, lo = _split_pack(w_t, O_SHARD)
    return np.ascontiguousarray(np.stack([hi, lo], axis=1))


def kernel(x: np.ndarray, weights: np.ndarray, bias: np.ndarray) -> np.ndarray:
    from concourse.bass_utils import run_bass_kernel_spmd

    if "nc" not in _STATE:
        _STATE["nc"] = _build()
    nc = _STATE["nc"]

    x = np.asarray(x, dtype=np.float32)
    weights = np.asarray(weights, dtype=np.float32)
    bias = np.asarray(bias, dtype=np.float32)

    xt = np.ascontiguousarray(x.T)  # [SIZE_IN, BATCH] f32
    xh, xl = _split_pack(xt, BATCH)
    wt = np.ascontiguousarray(weights.T)  # [SIZE_IN, SIZE_OUT] f32

    in_maps = []
    for c in range(N_CORES):
        sl = slice(c * O_SHARD, (c + 1) * O_SHARD)
        in_maps.append(
            {
                "xh": xh,
                "xl": xl,
                "whl": _split_pack_w(np.ascontiguousarray(wt[:, sl])),
                "bias": np.ascontiguousarray(bias[sl]).reshape(O_SHARD, 1),
            }
        )

    # Always install the shim: if BASS_TRACE is set in the environment,
    # run_bass_kernel_spmd imports antenv.axon_hooks unconditionally and
    # would otherwise crash on images whose antenv lacks that module.
    _install_ntff_hook_shim()
    trace = os.environ.get("BASS_PROBLEM_TRACE", "0") == "1"
    res = run_bass_kernel_spmd(
        nc, in_maps, core_ids=list(range(N_CORES)), trace=trace
    )
    _STATE["last_results"] = res

    out_t = np.concatenate(
        [np.asarray(res.results[c]["out"]) for c in range(N_CORES)], axis=0
    )  # [SIZE_OUT, BATCH]
    return np.ascontiguousarray(out_t.T).astype(np.float32, copy=False)


# revision 29
# speedup vs baseline: 1.1042x; 1.0422x over previous
"""Memristor linear layer kernel for 8 TRN2 NeuronCores.

The reference memristor crossbar computation collapses algebraically to
    out = x @ weights.T + bias
(the G_OFF offsets cancel in the pos/neg column subtraction and the k_G /
k_I scale factors cancel exactly), so the kernel computes the plain linear
layer.

Precision: fp32 operands are split on host into bf16 hi + bf16 lo halves;
the device computes hi*hi + hi*lo + lo*hi with fp32 PSUM accumulation
(~4e-6 relative error vs 3e-7 for native fp32) at full bf16 PE rate.

Sharding: tensor-parallel over the 1024 output features -> 128 per core.
Each core receives x.T (replicated) and its W.T column shard, pre-packed
on host into the exact SBUF layout [128 partitions, k_tile, free] so
every DMA moves per-partition-contiguous rows at line rate. Each core
computes its out.T shard [128, 256] = W_shard @ x.T + bias accumulated
over 8 K-chunks of 128 in PSUM. Host concatenates and transposes back.

Schedule notes (from NTFF profiling on TRN2 under axon):
- The HWDGE rings drain in global issue order at ~280 GB/s, with ~1 us
  per-transfer completion latency, so transfers are staged in the exact
  order the matmul passes need them (wh | xh halves, wl, xl halves).
- The PE HAM clock gate needs ~3.4 us of sustained busy-ness to release
  (1.2 -> 2.4 GHz) and re-throttles after ~2 us of idle, so garbage
  warm-up matmuls run while DMAs stream and tiny filler matmuls are
  interleaved between compute passes to bridge DMA chase-stalls.
"""

import os

import numpy as np

BATCH = 256
SIZE_IN = 1024
SIZE_OUT = 1024
N_CORES = 8
O_SHARD = SIZE_OUT // N_CORES  # 128
K_TILES = SIZE_IN // 128  # 8

_STATE = {}


def _build():
    import concourse.bass as bass
    import concourse.tile as tile
    from concourse import bacc, mybir

    f32 = mybir.dt.float32
    bf16 = mybir.dt.bfloat16
    n_warm = int(os.environ.get("WARMUP_MM", "5"))

    nc = bacc.Bacc(None, target_bir_lowering=False)

    # All tensors pre-packed on host to [128, ..., free] (partition major)
    # so every DMA descriptor is a large per-partition-contiguous run.
    xh_d = nc.declare_dram_parameter("xh", [128, K_TILES, BATCH], bf16, isOutput=False)
    xl_d = nc.declare_dram_parameter("xl", [128, K_TILES, BATCH], bf16, isOutput=False)
    whl_d = nc.declare_dram_parameter(
        "whl", [128, 2, K_TILES, O_SHARD], bf16, isOutput=False
    )
    b_d = nc.declare_dram_parameter("bias", [O_SHARD, 1], f32, isOutput=False)
    out_d = nc.declare_dram_parameter("out", [O_SHARD, BATCH], f32, isOutput=True)

    with tile.TileContext(nc) as tc:
        with (
            tc.tile_pool(name="sbuf", bufs=1) as pool,
            tc.tile_pool(name="psum", bufs=1, space="PSUM") as psum_pool,
        ):
            xh_s = pool.tile([128, K_TILES, BATCH], bf16)
            xl_s = pool.tile([128, K_TILES, BATCH], bf16)
            whl_s = pool.tile([128, 2, K_TILES, O_SHARD], bf16)
            b_s = pool.tile([O_SHARD, 1], f32)
            o_s = pool.tile([O_SHARD, BATCH], f32)
            pt = psum_pool.tile([O_SHARD, BATCH], f32)

            # PE warm-up: garbage matmuls into a scratch PSUM bank so the
            # HAM clock-gate releases (1.2 -> 2.4 GHz) while DMAs stream.
            # A few big ones build the busy window, then small (~54 ns)
            # ones keep PE occupied at fine granularity until real data
            # lands; more small ones are interleaved between the compute
            # passes below so DMA chase-stalls can't re-throttle the PE.
            n_warm_small = int(os.environ.get("WARMUP_MM_SMALL", "30"))
            warm_in = pool.tile([128, 512], bf16)
            warm_ps = psum_pool.tile([128, 512], f32)
            nc.vector.memset(warm_in[:], 0.0)

            def warm_big(n):
                for _ in range(n):
                    nc.tensor.matmul(
                        warm_ps[:], warm_in[:, 0:128], warm_in[:], start=True,
                        stop=True,
                    )

            def warm_small(n):
                for _ in range(n):
                    nc.tensor.matmul(
                        warm_ps[:, 0:64], warm_in[:, 0:128], warm_in[:, 0:64],
                        start=True, stop=True,
                    )

            warm_big(n_warm)
            warm_small(n_warm_small)

            # Fine-grained transfers. Each engine issues its own queue in
            # program order and the HWDGE drains in global issue-time
            # order, so keep everything whose order matters on the scalar
            # ring; sync carries only the two wh halves issued up front.
            h = K_TILES // 2
            variant = os.environ.get("DMA_VARIANT", "whl1")
            if variant == "whl1":
                # One early 512 KB weight transfer (hi+lo), then x hi and
                # x lo halves chase on the scalar ring. Both weight halves
                # are ready when the first x chunk lands, so the lo*hi
                # pass interleaves early and only hi*lo waits for x lo.
                nc.sync.dma_start(out=whl_s[:], in_=whl_d[:])
                nc.scalar.dma_start(out=xh_s[:, 0:h, :], in_=xh_d[:, 0:h, :])
                nc.scalar.dma_start(out=xh_s[:, h:, :], in_=xh_d[:, h:, :])
                nc.scalar.dma_start(out=xl_s[:, 0:h, :], in_=xl_d[:, 0:h, :])
                nc.scalar.dma_start(out=xl_s[:, h:, :], in_=xl_d[:, h:, :])
                plan = (
                    [(0, xh_s, k) for k in range(h)]
                    + [(1, xh_s, k) for k in range(h)]
                    + [None]
                    + [(0, xh_s, k) for k in range(h, K_TILES)]
                    + [(1, xh_s, k) for k in range(h, K_TILES)]
                    + [None]
                    + [(0, xl_s, k) for k in range(K_TILES)]
                )
            else:
                # wh | xh halves | wl | xl halves in need order
                nc.sync.dma_start(out=whl_s[:, 0, :, :], in_=whl_d[:, 0, :, :])
                nc.scalar.dma_start(out=xh_s[:, 0:h, :], in_=xh_d[:, 0:h, :])
                nc.scalar.dma_start(out=xh_s[:, h:, :], in_=xh_d[:, h:, :])
                nc.sync.dma_start(out=whl_s[:, 1, :, :], in_=whl_d[:, 1, :, :])
                nc.scalar.dma_start(out=xl_s[:, 0:h, :], in_=xl_d[:, 0:h, :])
                nc.scalar.dma_start(out=xl_s[:, h:, :], in_=xl_d[:, h:, :])
                plan = (
                    [(0, xh_s, k) for k in range(h)]
                    + [None]
                    + [(0, xh_s, k) for k in range(h, K_TILES)]
                    + [None]
                    + [(1, xh_s, k) for k in range(K_TILES)]
                    + [None]
                    + [(0, xl_s, k) for k in range(K_TILES)]
                )
            # bias via the gpsimd SWDGE path (off both HWDGE rings)
            nc.gpsimd.dma_start(out=b_s[:], in_=b_d[:])
            mm_plan = [p for p in plan if p is not None]
            i = 0
            for p in plan:
                if p is None:
                    warm_small(int(os.environ.get("WARMUP_MM_GAP", "8")))
                    continue
                hl, xs, k = p
                nc.tensor.matmul(
                    pt[:],
                    whl_s[:, hl, k, :],
                    xs[:, k, :],
                    start=(i == 0),
                    stop=(i == len(mm_plan) - 1),
                )
                i += 1

            nc.vector.tensor_scalar_add(out=o_s[:], in0=pt[:], scalar1=b_s[:])
            # out halves on both HWDGE rings so the completion receipts
            # (~1 us each to HBM) overlap
            nc.sync.dma_start(out=out_d[:, 0:BATCH // 2], in_=o_s[:, 0:BATCH // 2])
            nc.scalar.dma_start(out=out_d[:, BATCH // 2 :], in_=o_s[:, BATCH // 2 :])

    nc.compile()
    return nc


def _install_ntff_hook_shim():
    """The agent image's antenv lacks axon_hooks; recreate it so
    run_bass_kernel_spmd(trace=True) can capture NTFF profiles."""
    import sys
    import types

    if "antenv.axon_hooks" in sys.modules:
        return
    try:
        import antenv.axon_hooks  # noqa: F401  (real module exists)

        return
    except ImportError:
        pass
    mod = types.ModuleType("antenv.axon_hooks")
    mod._HOOK = None

    def set_axon_ntff_profile_hook(hook):
        mod._HOOK = hook

    def get_axon_ntff_profile_hook():
        return mod._HOOK

    mod.set_axon_ntff_profile_hook = set_axon_ntff_profile_hook
    mod.get_axon_ntff_profile_hook = get_axon_ntff_profile_hook
    sys.modules["antenv.axon_hooks"] = mod
    try:
        from trn_agent_boot.trn_boot import _ntff_profile_via_ctypes

        mod._HOOK = _ntff_profile_via_ctypes("/opt/axon/libaxon_pjrt.so")
    except Exception:
        pass


def _split_pack(a_t: np.ndarray, ncols: int):
    """[SIZE_IN, ncols] f32 -> two bf16 arrays packed as [128, K_TILES, ncols]."""
    import ml_dtypes

    hi = a_t.astype(ml_dtypes.bfloat16)
    lo = (a_t - hi.astype(np.float32)).astype(ml_dtypes.bfloat16)

    def pack(v):
        return np.ascontiguousarray(
            v.reshape(K_TILES, 128, ncols).transpose(1, 0, 2)
        )

    return pack(hi), pack(lo)


def _split_pack_w(w_t: np.ndarray):
    """[SIZE_IN, O_SHARD] f32 -> one bf16 array [128, 2, K_TILES, O_SHARD]
    holding the hi and lo halves contiguously per partition."""
    hi, lo = _split_pack(w_t, O_SHARD)
    return np.ascontiguousarray(np.stack([hi, lo], axis=1))


def kernel(x: np.ndarray, weights: np.ndarray, bias: np.ndarray) -> np.ndarray:
    from concourse.bass_utils import run_bass_kernel_spmd

    if "nc" not in _STATE:
        _STATE["nc"] = _build()
    nc = _STATE["nc"]

    x = np.asarray(x, dtype=np.float32)
    weights = np.asarray(weights, dtype=np.float32)
    bias = np.asarray(bias, dtype=np.float32)

    xt = np.ascontiguousarray(x.T)  # [SIZE_IN, BATCH] f32
    xh, xl = _split_pack(xt, BATCH)
    wt = np.ascontiguousarray(weights.T)  # [SIZE_IN, SIZE_OUT] f32

    in_maps = []
    for c in range(N_CORES):
        sl = slice(c * O_SHARD, (c + 1) * O_SHARD)
        in_maps.append(
            {
                "xh": xh,
                "xl": xl,
                "whl": _split_pack_w(np.ascontiguousarray(wt[:, sl])),
                "bias": np.ascontiguousarray(bias[sl]).reshape(O_SHARD, 1),
            }
        )

    # Always install the shim: if BASS_TRACE is set in the environment,
    # run_bass_kernel_spmd imports antenv.axon_hooks unconditionally and
    # would otherwise crash on images whose antenv lacks that module.
    _install_ntff_hook_shim()
    trace = os.environ.get("BASS_PROBLEM_TRACE", "0") == "1"
    res = run_bass_kernel_spmd(
        nc, in_maps, core_ids=list(range(N_CORES)), trace=trace
    )
    _STATE["last_results"] = res

    out_t = np.concatenate(
        [np.asarray(res.results[c]["out"]) for c in range(N_CORES)], axis=0
    )  # [SIZE_OUT, BATCH]
    return np.ascontiguousarray(out_t.T).astype(np.float32, copy=False)
